# revision 20
# baseline (speedup 1.0000x reference)
# kernel.py — fused causal ReLU-attention (qkv proj + q@k^T + relu/causal + @v)
# for Trainium2, 8 NeuronCores, batch-parallel (1 batch element per core).
#
# Self-contained: hardcodes shapes B,T,C = 8,1024,768, nh=12, hs=64.
import os
import sys

for p in ("/opt/trn_rl_repo", "/root/.axon_site", "/root/.axon_site/_ro/trn_rl_repo"):
    if os.path.isdir(p) and p not in sys.path:
        sys.path.append(p)

import numpy as np

import concourse.bass as bass
import concourse.mybir as mybir
import concourse.tile as tile
from concourse import bacc
from concourse import bass_utils
from concourse.masks import make_identity

F32 = mybir.dt.float32
F32R = mybir.dt.float32r
BF16 = mybir.dt.bfloat16
AF = mybir.ActivationFunctionType
ALU = mybir.AluOpType

B, T, C = 8, 1024, 768
NH, HS = 12, 64
SCALE = 1.0 / 8.0  # 1/sqrt(64)
P = 128
NT = T // P    # 8 t-tiles
KC = C // P    # 6 c-tiles (contraction)
NPAIR = NH // 2  # 6 head pairs
TCH = 512      # t1 chunk size
NCH = T // TCH  # 2 chunks


def r32(ap):
    return ap.bitcast(F32R)


def build_nc(n_cores=8):
    nc = bacc.Bacc("TRN2", target_bir_lowering=False, debug=False,
                   num_devices=n_cores)

    x_d = nc.dram_tensor("x", [T, C], BF16, kind="ExternalInput").ap()
    w_d = nc.dram_tensor("w", [C, 3 * C], BF16, kind="ExternalInput").ap()
    b_d = nc.dram_tensor("b", [3 * C], F32, kind="ExternalInput").ap()
    y_d = nc.dram_tensor("y", [T, C], BF16, kind="ExternalOutput").ap()

    with tile.TileContext(nc) as tc:
        _emit(nc, tc, x_d, w_d, b_d, y_d)

    nc.compile()
    return nc


def _emit(nc, tc, x_d, w_d, b_d, y_d):
    from contextlib import ExitStack

    with ExitStack() as ctx:
        pp = ctx.enter_context(tc.tile_pool(name="persist", bufs=1))
        wqk = ctx.enter_context(tc.tile_pool(name="wqk", bufs=1))
        xtp = ctx.enter_context(tc.tile_pool(name="xT", bufs=1))
        ybp = ctx.enter_context(tc.tile_pool(name="yblk", bufs=6))
        ap_pool = ctx.enter_context(tc.tile_pool(name="attp", bufs=24))
        yt_pool = ctx.enter_context(tc.tile_pool(name="yT", bufs=3))
        qps = ctx.enter_context(
            tc.tile_pool(name="qkv_psum", bufs=2, space="PSUM"))
        a_ps = ctx.enter_context(
            tc.tile_pool(name="att_psum", bufs=3, space="PSUM"))
        y_ps = ctx.enter_context(
            tc.tile_pool(name="y_psum", bufs=3, space="PSUM"))

        # ---- constants ----
        bqk = pp.tile([P, 12], F32, tag="bqk", name="bqk")
        nc.sync.dma_start(bqk[:], b_d[0:2 * C].rearrange("(a p) -> p a", p=P))

        bv_row = pp.tile([1, C], F32, tag="bvrow", name="bvrow")
        nc.sync.dma_start(bv_row[:],
                          b_d[2 * C:3 * C].rearrange("(o a) -> o a", o=1))
        bv = pp.tile([P, C], F32, tag="bv", name="bv")
        nc.gpsimd.partition_broadcast(bv[:], bv_row[0:1, :])

        # master relu/causal mask, pre-scaled by SCALE:
        #   cols [0,384) = 0 ; [384,512) = (col-384>=part ? SCALE : 0) ;
        #   [512,896) = SCALE
        mstr = pp.tile([P, 896], F32, tag="mstr", name="mstr")
        nc.gpsimd.memset(mstr[:, 0:384], 0.0)
        nc.gpsimd.memset(mstr[:, 384:896], SCALE)
        nc.gpsimd.affine_select(
            out=mstr[:, 384:512], in_=mstr[:, 384:512],
            compare_op=ALU.is_ge, fill=0.0, base=0,
            pattern=[[1, P]], channel_multiplier=-1)

        # ---- persistent activations (bf16) ----
        qkT = [pp.tile([P, T], BF16, tag=f"qkT{m}", name=f"qkT{m}")
               for m in range(12)]
        v_sb = [pp.tile([P, C], BF16, tag=f"v{i}", name=f"v{i}")
                for i in range(NT)]

        xT = [xtp.tile([P, T], BF16, tag=f"xT{k}", name=f"xT{k}")
              for k in range(KC)]
        w_sb = [wqk.tile([P, 2 * C], BF16, tag=f"w{k}", name=f"w{k}")
                for k in range(KC)]

        # ---- loads: x arrives TRANSPOSED via DMA-transpose (bf16) ----
        for k in range(KC):
            nc.sync.dma_start(xT[k][:], x_d[:, P * k:P * (k + 1)],
                              transpose=True)
        wv_list = []

        def emit_qk_group(m, t):
            ps = qps.tile([P, TCH], F32, tag="qkvps", name="qkvps")
            for k in range(KC):
                nc.tensor.matmul(
                    ps[:],
                    w_sb[k][:, P * m:P * (m + 1)],
                    xT[k][:, TCH * t:TCH * (t + 1)],
                    start=(k == 0), stop=(k == KC - 1))
            if m % 2 == 0:
                nc.scalar.activation(qkT[m][:, TCH * t:TCH * (t + 1)],
                                     ps[:], AF.Identity,
                                     bias=bqk[:, m:m + 1])
            else:
                nc.vector.tensor_scalar(
                    qkT[m][:, TCH * t:TCH * (t + 1)], ps[:],
                    bqk[:, m:m + 1], None, ALU.add)

        # ================= phase C: v-part =================
        with ExitStack() as c1:
            wvp = c1.enter_context(tc.tile_pool(name="wv", bufs=1))
            wv = [wvp.tile([P, C], BF16, tag=f"wv{k}", name=f"wv{k}")
                  for k in range(KC)]
            for k in range(KC):
                nc.sync.dma_start(wv[k][:],
                                  w_d[P * k:P * (k + 1), 2 * C:3 * C])
            for k in range(KC):
                nc.sync.dma_start(w_sb[k][:],
                                  w_d[P * k:P * (k + 1), 0:2 * C])

            for i in range(NT):
                for (n0, n1) in ((0, 512), (512, 768)):
                    ps = qps.tile([P, TCH], F32, tag="qkvps", name="qkvps")
                    for k in range(KC):
                        nc.tensor.matmul(
                            ps[:, 0:n1 - n0],
                            xT[k][:, P * i:P * (i + 1)],
                            wv[k][:, n0:n1],
                            start=(k == 0), stop=(k == KC - 1))
                    nc.vector.tensor_tensor(
                        v_sb[i][:, n0:n1], ps[:, 0:n1 - n0],
                        bv[:, n0:n1], ALU.add)

        def att_piece_params(c):
            c_lo = TCH * c
            out = []
            for r in range((TCH * (c + 1)) // P):
                t2_0 = P * r
                off = max(0, t2_0 - c_lo)
                offp = min(off, TCH - 256)   # widen tails to N>=256
                z = off - offp
                n = TCH - offp
                out.append((r, offp, z, n, t2_0 >= c_lo))
            return out

        for m in (0, 6):
            for t in range(NCH):
                emit_qk_group(m, t)

        # ======= attention: per pair; att(j) interleaved with qk(j+1) =======
        for j in range(NPAIR):
            qt, kt = qkT[j], qkT[6 + j]
            # both heads' yT in one [128, T] bf16 tile
            yT2 = yt_pool.tile([P, T], BF16, tag="yT", name="yT")

            qk_next = []
            if j + 1 < NPAIR:
                qk_next = [(m, t) for m in (j + 1, 7 + j)
                           for t in range(NCH)]
            att_work = [(c, p) for c in range(NCH)
                        for p in att_piece_params(c)]
            chunk_pieces = {0: [], 1: []}
            qi = 0
            for wi, (c, (r, offp, z, n, diag)) in enumerate(att_work):
                if qi < len(qk_next) and wi % 3 == 0:
                    emit_qk_group(*qk_next[qi]); qi += 1
                c_lo = TCH * c
                for hh in range(2):
                    h0 = 64 * hh
                    ps = a_ps.tile([P, TCH], F32, tag="aps", name="aps")
                    nc.tensor.matmul(
                        ps[:, 0:n],
                        kt[h0:h0 + 64, P * r:P * r + P],
                        qt[h0:h0 + 64, c_lo + offp:TCH * (c + 1)],
                        start=True, stop=True,
                        tile_position=(h0, 0))
                    at = ap_pool.tile([P, TCH], BF16, tag="attp",
                                      name="attp")
                    if diag:
                        nc.vector.scalar_tensor_tensor(
                            at[:, 0:n], ps[:, 0:n], 0.0,
                            mstr[:, 384 - z:384 - z + n],
                            ALU.max, ALU.mult)
                    else:
                        nc.scalar.activation(at[:, 0:n], ps[:, 0:n],
                                             AF.Relu, scale=SCALE)
                    chunk_pieces[c].append((r, hh, offp, n, at))

                # av for chunk c; col-packed pair per piece
                if (c == 0 and r == 3) or (c == 1 and r == 7):
                    c_hi = TCH * (c + 1)
                    rmax = c_hi // P
                    yp = [y_ps.tile([P, TCH], F32, tag="yps",
                                    name="yps") for _ in range(2)]
                    for (r2, hh, offp2, n2, at2) in chunk_pieces[c]:
                        h0 = 64 * hh
                        nc.tensor.matmul(
                            yp[hh][h0:h0 + 64, offp2:offp2 + n2],
                            v_sb[r2][:, P * j + h0:P * j + h0 + 64],
                            at2[:, 0:n2],
                            start=(r2 == 0), stop=(r2 == rmax - 1),
                            tile_position=(0, h0))
                    for hh in range(2):
                        h0 = 64 * hh
                        if (c + hh) % 2 == 0:
                            nc.scalar.activation(
                                yT2[h0:h0 + 64, c_lo:c_hi],
                                yp[hh][h0:h0 + 64, :], AF.Copy)
                        else:
                            nc.vector.tensor_copy(
                                yT2[h0:h0 + 64, c_lo:c_hi],
                                yp[hh][h0:h0 + 64, :])
            while qi < len(qk_next):
                emit_qk_group(*qk_next[qi]); qi += 1

            # ---- yT2 [hd, t] -> y [t, hd] via DMA-transpose (bf16) ----
            for i in range(NT):
                yb = ybp.tile([P, P], BF16, tag="yb", name="yb")
                nc.sync.dma_start(yb[:], yT2[:, P * i:P * (i + 1)],
                                  transpose=True)
                nc.sync.dma_start(
                    y_d[P * i:P * (i + 1), P * j:P * (j + 1)], yb[:])


def _ensure_ntff_hook():
    """Register the axon NTFF profiling hook if the image's antenv lacks
    axon_hooks (bass_utils hard-imports it on the trace=True path)."""
    import types
    try:
        from antenv import axon_hooks  # noqa: F401
        return
    except ImportError:
        pass
    import antenv
    mod = types.ModuleType("antenv.axon_hooks")
    mod._hook = None

    def set_axon_ntff_profile_hook(h):
        mod._hook = h

    def get_axon_ntff_profile_hook():
        return mod._hook

    mod.set_axon_ntff_profile_hook = set_axon_ntff_profile_hook
    mod.get_axon_ntff_profile_hook = get_axon_ntff_profile_hook
    sys.modules["antenv.axon_hooks"] = mod
    antenv.axon_hooks = mod
    try:
        from trn_agent_boot.trn_boot import _ntff_profile_via_ctypes
        hook = _ntff_profile_via_ctypes("/opt/axon/libaxon_pjrt.so")
        if hook is not None:
            mod._hook = hook
    except Exception:
        pass


_NC_CACHE = None


def _get_nc():
    global _NC_CACHE
    if _NC_CACHE is None:
        _NC_CACHE = build_nc()
    return _NC_CACHE


def kernel(x, W_attn, b_attn, _trace=False):
    import ml_dtypes
    x = np.ascontiguousarray(np.asarray(x).astype(ml_dtypes.bfloat16))
    w = np.ascontiguousarray(np.asarray(W_attn).astype(ml_dtypes.bfloat16))
    b = np.ascontiguousarray(np.asarray(b_attn, dtype=np.float32))
    assert x.shape == (B, T, C) and w.shape == (C, 3 * C) and b.shape == (3 * C,)

    if _trace:
        _ensure_ntff_hook()
    nc = _get_nc()
    in_maps = [{"x": x[i], "w": w, "b": b} for i in range(B)]
    res = bass_utils.run_bass_kernel_spmd(
        nc, in_maps, core_ids=list(range(B)), trace=_trace)
    y = np.stack([np.asarray(res.results[i]["y"]).astype(np.float32)
                  for i in range(B)], axis=0)
    if _trace:
        kernel.last_result = res
    return y
